# revision 19
# baseline (speedup 1.0000x reference)
"""Trainium2 Bass kernel for single-head attention.

Reference computation (per batch b):
    q = x @ Wq; k = x @ Wk; v = x @ Wv          # x: [S, D], W: [D, D]
    out = softmax(q @ k.T / sqrt(D)) @ v

Shapes: B=4, S=2048, D=1024, f32.
Sharding over 8 NeuronCores: core c -> (batch b = c//2, seq half h = c%2).

Dataflow (vs the v1 AllGather kernel, 337us -> ~291us):

* The pair exchange of k^T / v uses AllReduce(add); each core keeps its
  OWN k^T / v shard in SBUF and reconstructs the peer shard as
  (pair_sum - own) on DVE.  This is what lets an SPMD-uniform program
  address "the peer's data": the sum is rank-symmetric, while AllGather
  slots are indexed by source rank.  Key order per core is [own | peer]
  (softmax + attn@v are permutation-invariant over keys as long as attn
  columns and v rows agree).  k^T is exchanged in fp16 (halves the wire
  time of the 2-rank mesh collective, ~35 GB/s; logit noise ~0.4 units,
  measured end-to-end rel err 0.012 vs the 2e-2 gate); v in bf16.
* That unlocks the overlap v1 lacked: attention-vs-own-keys (~33us of
  PE) runs while the collectives deliver the peer half.

Phases (PE stream):
  1. x -> xT transposes; projections kT -> v -> qT (own 1024 rows).
     kT/v halves stream to DRAM as produced; 2 pairwise AllReduce chunks
     each trigger as soon as their half is staged.  A tiny fp16-AR
     barrier fires at t~2us to absorb the ~30us ncfw warmup + first-AR
     setup and to align pair skew.  Collectives serialize on the CC
     core (each collective_compute also blocks gpsimd until done), so
     gpsimd carries only the identb cast-DMA + triggers.
  2. Phase A (overlapped with collectives): scores vs OWN keys for all
     8 query tiles; per-tile rowmax m_own; scores stored rowmax-shifted
     in fp16 (values <= 0, so fp16 error vanishes near the max -- the
     only region softmax cares about).
  3. Phase B per query tile (pipelined sp(i+1) | T(i) | AV(i)):
     scores vs PEER keys (psum), m_full = max(m_own, m_peer), exp of
     both halves into bf16 attn (scalar engine, per-partition biases
     d_own = (m_own-m_full)/32 resp. -m_full/32), attn transpose
     (PE+DVE), attn @ v with own key-tiles first / peer last (so the
     late v collective only stalls the first AV's tail), scale by 1/l,
     bf16 out staged + DMA (host widens to f32).

dtype strategy: all matmuls f32r (full rate at free>=256), attn
weights / v / out in bf16, exchanged k^T in fp16.

SBUF is fully tag-recycled (~202KB/207.9KB):
  wk -> wq -> attn | wv -> kT_peer | xT -> shifted-scores |
  kT_own -> v_peer | x-staging -> kt-fp16-staging -> 3 early v_peer
(v_peer readback mostly lands only after kT_own dies at phase-A end;
the v AllReduce result waits in DRAM until then.)
"""

import numpy as np

import concourse.bass as bass
import concourse.mybir as mybir
import concourse.tile as tile
from concourse import bacc
from concourse.bass_utils import run_bass_kernel_spmd

P = 128          # partitions
D = 1024         # model dim (= E)
S_OWN = 1024     # sequence rows per core
S_FULL = 2048
B, NCORES = 4, 8
DT = D // P      # 8 d-tiles
ST = S_OWN // P  # 8 s-tiles (query tiles per core)
NT = S_FULL // P  # 16 key tiles
F32 = mybir.dt.float32
F32R = mybir.dt.float32r
BF16 = mybir.dt.bfloat16
FP16 = mybir.dt.float16
REPLICA_GROUPS = [[0, 1], [2, 3], [4, 5], [6, 7]]
KT_CHUNKS = 2    # k^T AllReduce: fp16 chunks (1MB each)
V_CHUNKS = 2     # v AllReduce: bf16 chunks (1MB each)


def build_kernel():
    nc = bacc.Bacc("TRN2", target_bir_lowering=False, num_devices=NCORES)

    x_d = nc.dram_tensor("x", [S_OWN, D], F32, kind="ExternalInput")
    wq_d = nc.dram_tensor("Wq", [D, D], F32, kind="ExternalInput")
    wk_d = nc.dram_tensor("Wk", [D, D], F32, kind="ExternalInput")
    wv_d = nc.dram_tensor("Wv", [D, D], F32, kind="ExternalInput")
    out_d = nc.dram_tensor("out", [S_OWN, D], BF16, kind="ExternalOutput")

    # collective bounce buffers (internal DRAM), chunked along e/s
    ec = D // KT_CHUNKS   # e-rows per kT chunk
    sc = S_OWN // V_CHUNKS
    kt_send = [nc.dram_tensor(f"kt_send{i}", [ec, S_OWN], FP16)
               for i in range(KT_CHUNKS)]
    kt_sum = [nc.dram_tensor(f"kt_sum{i}", [ec, S_OWN], FP16)
              for i in range(KT_CHUNKS)]
    v_send = [nc.dram_tensor(f"v_send{i}", [sc, D], BF16)
              for i in range(V_CHUNKS)]
    v_sum = [nc.dram_tensor(f"v_sum{i}", [sc, D], BF16)
             for i in range(V_CHUNKS)]

    bar_send = nc.dram_tensor("bar_send", [1, 128], FP16)
    bar_out = nc.dram_tensor("bar_out", [1, 128], FP16)

    ident_np = np.eye(P, dtype=np.float32)
    ident_d = nc.inline_tensor(ident_np, name="ident")

    with tile.TileContext(nc) as tc:
        _emit(nc, tc, x_d, wq_d, wk_d, wv_d, out_d,
              kt_send, kt_sum, v_send, v_sum, ident_d, bar_send, bar_out)
    nc.compile()
    return nc


def _emit(nc, tc, x_d, wq_d, wk_d, wv_d, out_d,
          kt_send, kt_sum, v_send, v_sum, ident_d, bar_send, bar_out):
    with tc.tile_pool(name="sb", bufs=1) as sb:
        # pair barrier: a tiny fp16 AllReduce absorbs both the ~30us ncfw
        # warmup AND the first-AllReduce setup cost (~16-47us) that would
        # otherwise delay the k^T collective.  Each collective_compute
        # blocks the gpsimd engine until completion, so gpsimd carries
        # only the identb cast-DMA (emitted first) plus the triggers.
        identb = sb.tile([P, P], BF16, name="identb")
        nc.gpsimd.dma_start(identb[:], ident_d.ap())  # cast f32->bf16
        nc.gpsimd.dma_start(bar_send.ap(), ident_d.ap()[0:1, 0:64].bitcast(FP16))
        nc.gpsimd.collective_compute(
            "AllReduce", mybir.AluOpType.add,
            replica_groups=REPLICA_GROUPS,
            ins=[bar_send.ap().opt()],
            outs=[bar_out.ap().opt()],
        )
        ident = sb.tile([P, P], F32, name="ident")
        nc.sync.dma_start(ident[:], ident_d.ap())

        # ---- SBUF tag plan (4KB/partition slots, generational reuse) ----
        # T1 "wk": wk -> wq -> attn      (wk dies @kT-proj; wq loads then,
        #                                 dies @qT-proj; attn in phase B)
        # T2 "wv": wv -> ktp             (wv dies @v-proj; peer kT lands)
        # T3 "xT": xT -> t_own fp16      (xT dies @qT-proj end)
        # T4 "kt": kT_own -> vp          (kT_own dies @phase-A end; the
        #                                 deferred v readback then lands)
        # T5 "qT", T6 "vo": v_own, T7 "xa": x staging, T8/T9 transients
        wk_sb = [sb.tile([P, D], F32R, name=f"wk{d}", tag="wk", bufs=8)
                 for d in range(DT)]
        wv_sb = [sb.tile([P, D], F32R, name=f"wv{d}", tag="wv", bufs=8)
                 for d in range(DT)]
        xT = [sb.tile([P, S_OWN], F32R, name=f"xT{d}", tag="xT", bufs=8)
              for d in range(DT)]
        kT_own = [sb.tile([P, S_OWN], F32R, name=f"kTo{e}", tag="kt", bufs=8)
                  for e in range(DT)]
        v_own = [sb.tile([P, D], BF16, name=f"vo{s}", tag="vo", bufs=8)
                 for s in range(ST)]
        qT = [sb.tile([P, S_OWN], F32R, name=f"qT{e}", tag="qT", bufs=8)
              for e in range(DT)]

        # per-qt softmax stats
        m_own = sb.tile([P, ST], F32, name="m_own", tag="stats")

        with tc.tile_pool(name="ps1", bufs=1, space="PSUM") as ps1:
            # ---- load x (sync queue), transpose to xT (f32r) ----
            x_nats = []
            for s in range(ST):
                x_nat = sb.tile([P, D], F32, name=f"x_nat{s}", tag="xa",
                                bufs=3)
                nc.sync.dma_start(x_nat[:], x_d.ap()[s * P:(s + 1) * P, :])
                x_nats.append(x_nat)
            # weight loads on the scalar HWDGE queue (f32 -> f32r bitcast);
            # wq reuses wk's slots, so its DMA waits for kT-proj to finish
            for d in range(DT):
                nc.scalar.dma_start(
                    wk_sb[d][:], wk_d.ap()[d * P:(d + 1) * P, :].bitcast(F32R))
            for d in range(DT):
                nc.scalar.dma_start(
                    wv_sb[d][:], wv_d.ap()[d * P:(d + 1) * P, :].bitcast(F32R))
            wq_sb = [sb.tile([P, D], F32R, name=f"wq{d}", tag="wk", bufs=8)
                     for d in range(DT)]
            for d in range(DT):
                nc.scalar.dma_start(
                    wq_sb[d][:], wq_d.ap()[d * P:(d + 1) * P, :].bitcast(F32R))
            for s in range(ST):
                x_nat = x_nats[s]
                for d in range(DT):
                    pt = ps1.tile([P, P], F32, name=f"pt{s}_{d}", tag="pt",
                                  bufs=2)
                    nc.tensor.transpose(pt[:], x_nat[:, d * P:(d + 1) * P],
                                        ident[:])
                    nc.vector.tensor_copy(xT[d][:, s * P:(s + 1) * P], pt[:])

            # ---- k^T projection -> SBUF (kept) + DRAM -> chunked AllReduce
            epc = DT // KT_CHUNKS
            for ch in range(KT_CHUNKS):
                for ei in range(epc):
                    e = ch * epc + ei
                    pk = ps1.tile([P, S_OWN], F32, name=f"pk{e}", tag="proj",
                                  bufs=3)
                    for d in range(DT):
                        for c in range(2):
                            nc.tensor.matmul(
                                pk[:, c * 512:(c + 1) * 512],
                                wk_sb[d][:, e * P:(e + 1) * P],
                                xT[d][:, c * 512:(c + 1) * 512],
                                start=(d == 0), stop=(d == DT - 1))
                    nc.vector.tensor_copy(kT_own[e][:], pk[:])
                    kts16 = sb.tile([P, S_OWN], FP16, name=f"kts16_{e}",
                                    tag="xa", bufs=3)
                    nc.scalar.copy(kts16[:], pk[:])
                    nc.sync.dma_start(
                        kt_send[ch].ap()[ei * P:(ei + 1) * P, :], kts16[:])
                nc.gpsimd.collective_compute(
                    "AllReduce", mybir.AluOpType.add,
                    replica_groups=REPLICA_GROUPS,
                    ins=[kt_send[ch].ap().opt()],
                    outs=[kt_sum[ch].ap().opt()],
                )

            # ---- v projection -> SBUF (kept, bf16) + DRAM -> AllReduce ----
            spc = ST // V_CHUNKS
            for ch in range(V_CHUNKS):
                for si in range(spc):
                    s = ch * spc + si
                    pv = ps1.tile([P, D], F32, name=f"pv{s}", tag="proj",
                                  bufs=3)
                    for d in range(DT):
                        for c in range(2):
                            nc.tensor.matmul(
                                pv[:, c * 512:(c + 1) * 512],
                                xT[d][:, s * P:(s + 1) * P],
                                wv_sb[d][:, c * 512:(c + 1) * 512],
                                start=(d == 0), stop=(d == DT - 1))
                    nc.vector.tensor_copy(v_own[s][:], pv[:])
                    nc.sync.dma_start(v_send[ch].ap()[si * P:(si + 1) * P, :],
                                      v_own[s][:])
                nc.gpsimd.collective_compute(
                    "AllReduce", mybir.AluOpType.add,
                    replica_groups=REPLICA_GROUPS,
                    ins=[v_send[ch].ap().opt()],
                    outs=[v_sum[ch].ap().opt()],
                )

            # ---- q^T projection ----
            for e in range(DT):
                pq = ps1.tile([P, S_OWN], F32, name=f"pq{e}", tag="proj",
                              bufs=3)
                for d in range(DT):
                    for c in range(2):
                        nc.tensor.matmul(
                            pq[:, c * 512:(c + 1) * 512],
                            wq_sb[d][:, e * P:(e + 1) * P],
                            xT[d][:, c * 512:(c + 1) * 512],
                            start=(d == 0), stop=(d == DT - 1))
                nc.vector.tensor_copy(qT[e][:], pq[:])

            # ---- peer kT readback + reconstruct (sum - own), during A ----
            ktp = [sb.tile([P, S_OWN], F32R, name=f"ktp{e}", tag="wv", bufs=8)
                   for e in range(DT)]
            for ch in range(KT_CHUNKS):
                for ei in range(epc):
                    e = ch * epc + ei
                    ktp16 = sb.tile([P, S_OWN], FP16, name=f"ktp16_{e}",
                                    tag="xa", bufs=3)
                    nc.scalar.dma_start(
                        ktp16[:], kt_sum[ch].ap()[ei * P:(ei + 1) * P, :])
                    nc.vector.tensor_sub(ktp[e][:], ktp16[:],
                                         kT_own[e][:].bitcast(F32))

            # ---- Phase A: scores vs OWN keys; rowmax-shifted fp16 store ----
            t_own = [sb.tile([P, S_OWN], FP16, name=f"town{q}", tag="xT",
                             bufs=8)
                     for q in range(ST)]
            negm_own = sb.tile([P, ST], F32, name="negm_own", tag="stats2")
            for sq in range(ST):
                so = ps1.tile([P, S_OWN], F32, name=f"so{sq}", tag="proj",
                              bufs=3)
                for e in range(DT):
                    for c in range(2):
                        nc.tensor.matmul(
                            so[:, c * 512:(c + 1) * 512],
                            qT[e][:, sq * P:(sq + 1) * P],
                            kT_own[e][:, c * 512:(c + 1) * 512],
                            start=(e == 0), stop=(e == DT - 1))
                nc.vector.reduce_max(m_own[:, sq:sq + 1], so[:],
                                     axis=mybir.AxisListType.X)
                nc.scalar.mul(negm_own[:, sq:sq + 1], m_own[:, sq:sq + 1],
                              -1.0 / 32.0)
                # t_own = (S - m_own)/32  (<= 0, fp16-exact near the max)
                nc.scalar.activation(
                    t_own[sq][:], so[:],
                    mybir.ActivationFunctionType.Identity,
                    bias=negm_own[:, sq:sq + 1], scale=1.0 / 32.0)

        # ---- deferred peer-v readback (lands once kT_own slots die) ----
        vp = [sb.tile([P, D], BF16, name=f"vp{s}",
                      tag=("xa" if s < 3 else "kt"), bufs=(3 if s < 3 else 8))
              for s in range(ST)]
        for ch in range(V_CHUNKS):
            for si in range(spc):
                s = ch * spc + si
                nc.scalar.dma_start(vp[s][:],
                                    v_sum[ch].ap()[si * P:(si + 1) * P, :])
        for s in range(ST):
            nc.vector.tensor_sub(vp[s][:], vp[s][:], v_own[s][:])

        # ---- Phase B: peer scores + softmax + attnT + attn@v ----
        with tc.tile_pool(name="ps2", bufs=1, space="PSUM") as ps2:
            state = {}

            def emit_sp(sq):
                sp = ps2.tile([P, S_OWN], F32, name=f"sp{sq}", tag="sp",
                              bufs=2)
                for e in range(DT):
                    for c in range(2):
                        nc.tensor.matmul(
                            sp[:, c * 512:(c + 1) * 512],
                            qT[e][:, sq * P:(sq + 1) * P],
                            ktp[e][:, c * 512:(c + 1) * 512],
                            start=(e == 0), stop=(e == DT - 1))
                state[sq] = sp

            def emit_exp(sq):
                sp = state.pop(sq)
                mp = sb.tile([P, 1], F32, name=f"mp{sq}", tag="mp", bufs=2)
                nc.vector.reduce_max(mp[:], sp[:], axis=mybir.AxisListType.X)
                mf = sb.tile([P, 1], F32, name=f"mf{sq}", tag="mf", bufs=2)
                nc.vector.tensor_max(mf[:], mp[:], m_own[:, sq:sq + 1])
                negm = sb.tile([P, 1], F32, name=f"negm{sq}", tag="negm",
                               bufs=2)
                nc.scalar.mul(negm[:], mf[:], -1.0 / 32.0)
                # d_own = (m_own - m_full)/32  (<= 0)
                d_own = sb.tile([P, 1], F32, name=f"down{sq}", tag="down",
                                bufs=2)
                nc.vector.tensor_scalar(
                    d_own[:], m_own[:, sq:sq + 1], mf[:, 0:1], 1.0 / 32.0,
                    mybir.AluOpType.subtract, mybir.AluOpType.mult)
                attn = sb.tile([P, S_FULL], BF16, name=f"attn{sq}", tag="wk",
                               bufs=8)
                l_own = sb.tile([P, 1], F32, name=f"lo{sq}", tag="lo", bufs=2)
                l_peer = sb.tile([P, 1], F32, name=f"lp{sq}", tag="lp",
                                 bufs=2)
                nc.scalar.activation(
                    attn[:, 0:S_OWN], t_own[sq][:],
                    mybir.ActivationFunctionType.Exp,
                    bias=d_own[:, 0:1], scale=1.0, accum_out=l_own[:])
                nc.scalar.activation(
                    attn[:, S_OWN:S_FULL], sp[:],
                    mybir.ActivationFunctionType.Exp,
                    bias=negm[:, 0:1], scale=1.0 / 32.0, accum_out=l_peer[:])
                rl = sb.tile([P, 1], F32, name=f"rl{sq}", tag="rl", bufs=2)
                lsum = sb.tile([P, 1], F32, name=f"ls{sq}", tag="ls", bufs=2)
                nc.vector.tensor_add(lsum[:], l_own[:], l_peer[:])
                nc.vector.reciprocal(rl[:], lsum[:])
                state[(sq, "sm")] = (attn, rl)

            def emit_T(sq):
                attn, rl = state[(sq, "sm")]
                attnT = sb.tile([P, S_FULL], BF16, name=f"attnT{sq}",
                                tag="attnT", bufs=2)
                for t in range(NT):
                    pat = ps2.tile([P, P], BF16, name=f"pat{sq}_{t}",
                                   tag="pat", bufs=2)
                    nc.tensor.transpose(
                        pat[:], attn[:, t * P:(t + 1) * P], identb[:])
                    nc.vector.tensor_copy(attnT[:, t * P:(t + 1) * P], pat[:])
                state[(sq, "T")] = attnT

            def emit_AV(sq):
                attn, rl = state.pop((sq, "sm"))
                attnT = state.pop((sq, "T"))
                O_ps = ps2.tile([P, D], F32, name=f"O{sq}", tag="O", bufs=1)
                for t in range(NT):
                    v_src = v_own[t] if t < ST else vp[t - ST]
                    for c in range(2):
                        nc.tensor.matmul(
                            O_ps[:, c * 512:(c + 1) * 512],
                            attnT[:, t * P:(t + 1) * P],
                            v_src[:, c * 512:(c + 1) * 512],
                            start=(t == 0), stop=(t == NT - 1))
                o_stage = sb.tile([P, D], BF16, name=f"ost{sq}", tag="ost",
                                  bufs=1)
                nc.vector.tensor_scalar_mul(o_stage[:], O_ps[:], rl[:, 0:1])
                nc.sync.dma_start(out_d.ap()[sq * P:(sq + 1) * P, :],
                                  o_stage[:])

            # PE stream: sp(0) | sp(1) T(0) AV(0) | sp(2) T(1) AV(1) | ...
            emit_sp(0)
            emit_exp(0)
            for sq in range(1, ST):
                emit_sp(sq)
                emit_T(sq - 1)
                emit_AV(sq - 1)
                emit_exp(sq)
            emit_T(ST - 1)
            emit_AV(ST - 1)


_NC_CACHE = {}


def _get_nc():
    if "nc" not in _NC_CACHE:
        _NC_CACHE["nc"] = build_kernel()
    return _NC_CACHE["nc"]


def kernel(x, Wq, Wk, Wv, **_ignored):
    x = np.ascontiguousarray(np.asarray(x, dtype=np.float32))
    Wq = np.ascontiguousarray(np.asarray(Wq, dtype=np.float32))
    Wk = np.ascontiguousarray(np.asarray(Wk, dtype=np.float32))
    Wv = np.ascontiguousarray(np.asarray(Wv, dtype=np.float32))
    nc = _get_nc()
    in_maps = []
    for c in range(NCORES):
        b, h = divmod(c, 2)
        in_maps.append({
            "x": x[b, h * S_OWN:(h + 1) * S_OWN, :],
            "Wq": Wq, "Wk": Wk, "Wv": Wv,
        })
    res = run_bass_kernel_spmd(nc, in_maps, core_ids=list(range(NCORES)))
    out = np.empty((B, S_FULL, D), dtype=np.float32)
    for c in range(NCORES):
        b, h = divmod(c, 2)
        out[b, h * S_OWN:(h + 1) * S_OWN, :] = np.asarray(
            res.results[c]["out"], dtype=np.float32)
    return out
